# revision 6
# baseline (speedup 1.0000x reference)
"""Trainium2 Bass kernel for nn_DiffForest (soft decision forest forward).

Math: per tree t, z = x @ w_d[t]; p = sigmoid(z); leaf path probs are products
of 8 factors p/(1-p) down a depth-8 tree; output = sum_t leaf_prob @ softmax(w_l[t]) / 10.

Kernel formulation (all on device except small weight prep):
  - The 512 "leaves" come in identical pairs -> fold to 256 paths; fold the
    pair-sum + 1/n_trees into the leaf weight matrix w2 (host, exact).
  - Path products move to log space:  -log P[q] = sum_path softplus(-z) + sum_{branch=1} z
    which is a matmul with a constant matrix S [512, 256]:
        A = S^T @ [softplus(-z); 32*z],   leaf_prob^T = exp(-A)   ([256 paths, batch])
    softplus(-z) = ln(1 + exp(-z)) via the Exp/Ln activation tables.
  - The decision matmul runs in fp8 e4m3 with DoubleRow perf mode (2x PE
    throughput): w_d is scaled by 32 on host so its values sit in e4m3's
    normal range; the 1/32 is folded into the Exp activation scale and into
    the z-entries of S (exact powers of two). G and the S-matmul are bf16
    (S entries are 1 and 1/32, exact); the leaf matmul is bf16 -- fp8 there
    costs too much accuracy.
  - Schedule: per batch chunk, [dec matmuls (all 10 trees)] then [leaf matmul
    of the PREVIOUS chunk] then [S matmuls]; the leaf block keeps the PE busy
    while the ACT engine swaps tables and runs the Ln block, so the S matmuls
    never stall on activations.
  - Sharding: data-parallel over batch; each of the 8 cores takes 2048 rows of x,
    weights replicated, no collectives.
"""

import numpy as np
import ml_dtypes

import concourse.bacc as bacc
import concourse.mybir as mybir
import concourse.tile as tile
from concourse.tile import add_dep_helper
from concourse.bass_utils import run_bass_kernel_spmd

N_CORES = 8
BATCH = 16384
B_LOC = BATCH // N_CORES        # 2048 rows per core
IN_DIM = 2048
N_TREES = 10
ND_PAD = 256                    # decision nodes padded 255 -> 256
NQ = 256                        # folded path (leaf) count
CLASSES = 1000
CHUNK = 512                     # batch columns processed per chunk
KI = IN_DIM // 128              # 16 contraction subtiles of 128
NDR = KI // 2                   # 8 DoubleRow steps (K=256 each)
W_SCALE = 32.0                  # host scale on w_d for the fp8 cast

BF16 = mybir.dt.bfloat16
F32 = mybir.dt.float32
F16 = mybir.dt.float16
FP8 = mybir.dt.float8e4
AF = mybir.ActivationFunctionType
DR = mybir.MatmulPerfMode.DoubleRow

_CACHE = {}


def _build(b_loc=B_LOC, n_trees=N_TREES):
    n_chunks = b_loc // CHUNK
    nc = bacc.Bacc("TRN2", target_bir_lowering=False)
    xt = nc.dram_tensor("xt", (IN_DIM, b_loc), FP8, kind="ExternalInput")
    wd = nc.dram_tensor("wd", (n_trees, IN_DIM, ND_PAD), FP8, kind="ExternalInput")
    smat = nc.dram_tensor("smat", (512, NQ), BF16, kind="ExternalInput")
    w2 = nc.dram_tensor("w2", (n_trees, NQ, CLASSES), BF16, kind="ExternalInput")
    out = nc.dram_tensor("out", (b_loc, CLASSES), F32, kind="ExternalOutput")

    with tile.TileContext(nc) as tc:
        with (
            tc.tile_pool(name="const", bufs=1) as constp,
            tc.tile_pool(name="sb", bufs=2) as sb,
            tc.tile_pool(name="wdp", bufs=3) as wdp,
            tc.tile_pool(name="ep", bufs=n_trees) as ep,
            tc.tile_pool(name="gp", bufs=n_trees) as gp,
            tc.tile_pool(name="outp", bufs=2) as outp,
            tc.tile_pool(name="lptp", bufs=2) as lptp,
            tc.tile_pool(name="pz", bufs=2, space="PSUM") as pzp,
            tc.tile_pool(name="plp", bufs=2, space="PSUM") as plpp,
            tc.tile_pool(name="po", bufs=2, space="PSUM") as pop,
        ):
            smat_sb = constp.tile([128, 4, NQ], BF16)
            w2_sb = constp.tile([128, n_trees, 2, CLASSES], BF16)

            first_mm = [None]
            started = False

            def emit_mm2(ci, lpT):
                c0 = ci * CHUNK
                for s in range(CHUNK // 128):
                    po = pop.tile([128, 1024], F32, tag="po")
                    n_acc = n_trees * 2
                    i = 0
                    for t in range(n_trees):
                        for lt in range(2):
                            first = i == 0
                            last = i == n_acc - 1
                            lhsT = lpT[:, t, lt, s * 128 : (s + 1) * 128]
                            nc.tensor.matmul(
                                po[:, 0:500], lhsT, w2_sb[:, t, lt, 0:500],
                                start=first, stop=last,
                            )
                            nc.tensor.matmul(
                                po[:, 512:1012], lhsT, w2_sb[:, t, lt, 500:1000],
                                start=first, stop=last,
                            )
                            i += 1
                    osb = outp.tile([128, CLASSES], F32, tag="osb")
                    nc.vector.tensor_copy(osb[:, 0:500], po[:, 0:500])
                    nc.vector.tensor_copy(osb[:, 500:1000], po[:, 512:1012])
                    nc.sync.dma_start(
                        out[c0 + s * 128 : c0 + (s + 1) * 128, :], osb[:, :]
                    )

            lpT_prev = None
            wd0_pieces = []
            for ci in range(n_chunks):
                c0 = ci * CHUNK
                xt_pieces = []
                for kq in range(4):
                    if ci == 0:
                        # interleave the first tree's wd pieces with the xt
                        # pieces so the first matmul's inputs issue first
                        wp = constp.tile([128, 4, ND_PAD], FP8, tag=f"wd0p{kq}")
                        nc.sync.dma_start(
                            wp[:, :, :],
                            wd[
                                0, 4 * kq * 128 : 4 * (kq + 1) * 128, :
                            ].rearrange("(k p) d -> p k d", p=128),
                        )
                        wd0_pieces.append(wp)
                    xp = sb.tile([128, 4, CHUNK], FP8, tag=f"xt{kq}")
                    nc.sync.dma_start(
                        xp[:, :, :],
                        xt[
                            4 * kq * 128 : 4 * (kq + 1) * 128, c0 : c0 + CHUNK
                        ].rearrange("(k p) n -> p k n", p=128),
                    )
                    xt_pieces.append(xp)

                # ---- decision matmuls (+ Exp on ACT), then Ln blocks ----
                # For ci==0 there is no previous-chunk leaf matmul to keep the
                # PE busy while the ACT engine swaps tables and runs Lns, so
                # split the trees into two phases: phase A's Ln block overlaps
                # phase B's decision matmuls.
                all_G = {}
                all_E = {}
                phases = [(0, 3), (3, n_trees)] if ci == 0 else [(0, n_trees)]

                def emit_dec_tree(t):
                    wd_pieces = None
                    wd_sb = None
                    if ci == 0 and t == 0:
                        wd_pieces = wd0_pieces
                    else:
                        wd_sb = wdp.tile([128, KI, ND_PAD], FP8, tag="wd")
                        wd_dma = nc.sync.dma_start(
                            wd_sb[:, :, :],
                            wd[t, :, :].rearrange("(k p) d -> p k d", p=128),
                        )
                        if ci == 0 and t in (1, 2):
                            add_dep_helper(
                                wd_dma.ins, first_mm[0].ins, sync=True,
                                reason="startup: critical pieces first",
                            )
                    G = gp.tile([128, 4, CHUNK], BF16, tag="G")
                    E = ep.tile([128, 2, CHUNK], F16, tag="E")
                    all_G[t] = G
                    all_E[t] = E
                    last_exp = None
                    for dt_ in range(2):
                        psz = pzp.tile([128, CHUNK], F32, tag="psz")
                        for j in range(NDR):
                            if wd_sb is None:
                                lhsT = wd_pieces[j // 2][
                                    :,
                                    2 * (j % 2) : 2 * (j % 2) + 2,
                                    dt_ * 128 : (dt_ + 1) * 128,
                                ]
                            else:
                                lhsT = wd_sb[
                                    :, 2 * j : 2 * j + 2,
                                    dt_ * 128 : (dt_ + 1) * 128,
                                ]
                            rhs = xt_pieces[j // 2][
                                :, 2 * (j % 2) : 2 * (j % 2) + 2, :
                            ]
                            mm = nc.tensor.matmul(
                                psz[:, :],
                                lhsT,
                                rhs,
                                start=(j == 0),
                                stop=(j == NDR - 1),
                                perf_mode=DR,
                            )
                            if first_mm[0] is None:
                                first_mm[0] = mm
                        # psz holds 32*z.  E = exp(-z) via scale=-1/32;
                        # Exp heads the ACT critical chain
                        last_exp = nc.scalar.activation(
                            E[:, dt_, :], psz[:, :], AF.Exp,
                            scale=-1.0 / W_SCALE,
                        )
                        nc.vector.tensor_copy(G[:, 2 + dt_, :], psz[:, :])
                    return last_exp

                for pi, (ta, tb) in enumerate(phases):
                    last_exp = None
                    for t in range(ta, tb):
                        last_exp = emit_dec_tree(t)
                    # leaf matmul of the previous chunk keeps PE busy while
                    # the ACT engine swaps tables and runs the Ln block
                    if lpT_prev is not None and pi == len(phases) - 1:
                        emit_mm2(ci - 1, lpT_prev)
                    # softplus(-z) = ln(exp(-z)+1); gate Lns on the last Exp
                    # so the ACT engine runs one Exp block then one Ln block
                    for t in range(ta, tb):
                        for dt_ in range(2):
                            ln = nc.scalar.activation(
                                all_G[t][:, dt_, :],
                                all_E[t][:, dt_, :],
                                AF.Ln,
                                bias=1.0,
                            )
                            add_dep_helper(
                                ln.ins, last_exp.ins, sync=False,
                                reason="batch ACT Ln block after Exp block",
                            )
                if not started:
                    nc.sync.dma_start(
                        smat_sb[:, :, :],
                        smat[:, :].rearrange("(k p) q -> p k q", p=128),
                    )
                    for t in range(n_trees):
                        nc.sync.dma_start(
                            w2_sb[:, t, :, :],
                            w2[t, :, :].rearrange("(l p) c -> p l c", p=128),
                        )
                    started = True

                # ---- S matmuls + lp exp ----
                lpT = lptp.tile([128, n_trees, 2, CHUNK], BF16, tag="lpT")
                for t in range(n_trees):
                    for lt in range(2):
                        plp = plpp.tile([128, CHUNK], F32, tag="plp")
                        for k in range(4):
                            nc.tensor.matmul(
                                plp[:, :],
                                smat_sb[:, k, lt * 128 : (lt + 1) * 128],
                                all_G[t][:, k, :],
                                start=(k == 0),
                                stop=(k == 3),
                            )
                        nc.scalar.activation(
                            lpT[:, t, lt, :], plp[:, :], AF.Exp, scale=-1.0
                        )
                lpT_prev = lpT
            emit_mm2(n_chunks - 1, lpT_prev)
    nc.compile()
    return nc


def _smat_np():
    # z-entries carry 1/W_SCALE: the PSUM holds 32*z and S undoes the scale
    S = np.zeros((512, NQ), np.float32)
    q = np.arange(NQ)
    zs = np.float32(1.0 / W_SCALE)
    for n in range(8):
        node = (2**n - 1) + (q >> (8 - n))
        branch = (q >> (7 - n)) & 1
        S[node, q] += 1.0
        S[256 + node, q] += branch.astype(np.float32) * zs
    return S.astype(ml_dtypes.bfloat16)


def _prep_weights(w_d, w_l, n_trees=N_TREES):
    bf16 = ml_dtypes.bfloat16
    fp8 = ml_dtypes.float8_e4m3
    w_l = np.asarray(w_l, dtype=np.float32)
    m = w_l.max(axis=-1, keepdims=True)
    e = np.exp(w_l - m, dtype=np.float32)
    sm = e / e.sum(axis=-1, keepdims=True)
    w2 = ((sm[:, 0::2, :] + sm[:, 1::2, :]) * np.float32(1.0 / n_trees)).astype(bf16)
    wd_p = np.zeros((n_trees, IN_DIM, ND_PAD), np.float32)
    wd_p[:, :, : w_d.shape[2]] = w_d
    wd_p *= np.float32(W_SCALE)
    return wd_p.astype(fp8), _smat_np(), w2


last_bass_results = None


def kernel(x, w_d, w_l):
    global last_bass_results
    x = np.asarray(x)
    wd_8, S, w2 = _prep_weights(np.asarray(w_d), np.asarray(w_l))
    x_8 = x.astype(ml_dtypes.float8_e4m3)
    in_maps = []
    for c in range(N_CORES):
        xt = np.ascontiguousarray(x_8[c * B_LOC : (c + 1) * B_LOC, :].T)
        in_maps.append({"xt": xt, "wd": wd_8, "smat": S, "w2": w2})
    if "nc" not in _CACHE:
        _CACHE["nc"] = _build()
    res = run_bass_kernel_spmd(_CACHE["nc"], in_maps, core_ids=list(range(N_CORES)))
    last_bass_results = res
    return np.concatenate([res.results[c]["out"] for c in range(N_CORES)], axis=0)
